# revision 1
# baseline (speedup 1.0000x reference)
"""Trainium2 Bass kernel for nn_BaselineBlockNetMultiGraph (single launch).

Sharding: data-parallel over batch (4 batches/core) for the GRU, adjacency
generation and the three GCN+conv blocks; l_out is tensor-parallel
(column-split over the flattened feature dim, 16384 cols/core). Features are
exchanged on-device with two batch-split AllToAll collectives pipelined
against the last conv block and the activation transposes; the host sums the
8 partial outputs (undoing a static batch permutation) and adds the bias.

Key implementation notes:
- GRU: (gate x series) layout, two independent 128-series half-chains; all
  matmul operands bf16 (PSUM accumulation groups must be dtype-uniform or
  the core hard-faults), elementwise gate math f32, x-input terms folded in
  as K=1 accumulating matmuls; off-chain elementwise on GPSIMD.
- Blocks: aggregate(stationary feats) -> theta -> temporal conv as shifted
  accumulating matmuls over a zero-padded buffer; one PE-transpose set per
  block; biases/LeakyReLU fused into PSUM evacuations, balanced ACT/DVE.
- l_out: 8-deep streamed bf16 weight pool (prefetch emitted early to avoid
  SP-queue head-of-line blocking); SBUF pools are temporally scoped so
  GRU/feats0 tenants die before the block phase needs the space.
- KTWOLAUNCH=1 selects a validated two-launch fallback with host-side
  feature exchange; KDEBUG=1 adds intermediate debug outputs.
"""

import os
import numpy as np
import ml_dtypes

import concourse.bass as bass
import concourse.mybir as mybir
import concourse.tile as tile
from concourse import bacc
from concourse.bass_utils import run_bass_kernel_spmd

B, T, N, C = 32, 32, 64, 64
GRU_H, QK, HOR = 64, 32, 12
KS = (3, 5, 7)
NCORES = 8
BL = B // NCORES            # 4 local batches per core
S = BL * N                  # 256 series per core
PAD = 3                     # max k//2
TSLOT = T + 2 * PAD         # 38 padded time slots
FEAT = N * C * T            # 131072
FSH = FEAT // NCORES        # 16384 feature cols per core
NR = N * HOR                # 768 output rows

F32 = mybir.dt.float32
BF16 = mybir.dt.bfloat16
BF = ml_dtypes.bfloat16
AF = mybir.ActivationFunctionType
ALU = mybir.AluOpType

DEBUG = bool(int(os.environ.get("KDEBUG", "0")))
GRUT = int(os.environ.get("KGRUT", str(T)))
NBLK = int(os.environ.get("KBLKS", "3"))

LAST_EXEC_NS = []
LAST_RESULTS = []


# ---------------------------------------------------------------- launch 1
def build_launch1(merged=False):
    nc = bacc.Bacc("TRN2", target_bir_lowering=False, num_devices=NCORES)
    if merged:
        wT = nc.dram_tensor("wT", [FSH, NR], BF16, kind="ExternalInput")
        partial = nc.dram_tensor("partial", [B, NR], F32, kind="ExternalOutput")

    xloc = nc.dram_tensor("xloc", [BL, T, N], F32, kind="ExternalInput")
    xlocb = nc.dram_tensor("xlocb", [BL, T, N], BF16, kind="ExternalInput")
    w_rz = nc.dram_tensor("w_rz", [65, 128], BF16, kind="ExternalInput")
    w_n = nc.dram_tensor("w_n", [64, 64], BF16, kind="ExternalInput")
    w_gin = nc.dram_tensor("w_gin", [1, 64], BF16, kind="ExternalInput")
    w_ih = nc.dram_tensor("w_ih", [1, 128], BF16, kind="ExternalInput")
    w_qk = nc.dram_tensor("w_qk", [64, 64], F32, kind="ExternalInput")
    gbias = nc.dram_tensor("gbias", [64, 4], F32, kind="ExternalInput")
    qkb = nc.dram_tensor("qkb", [QK, 2], F32, kind="ExternalInput")
    ident_f = nc.dram_tensor("ident_f", [128, 128], F32, kind="ExternalInput")
    ident_b = nc.dram_tensor("ident_b", [128, 128], BF16, kind="ExternalInput")
    m_x2i = nc.dram_tensor("m_x2i", [128, 64 * C], BF16, kind="ExternalInput")
    beta_row = nc.dram_tensor("beta_row", [1, 512], BF16, kind="ExternalInput")
    gcnw = nc.dram_tensor("gcnw", [3, T, C, C], BF16, kind="ExternalInput")
    gcnb = nc.dram_tensor("gcnb", [3, T, C], F32, kind="ExternalInput")
    cwt = nc.dram_tensor("cwt", [15, C, C], BF16, kind="ExternalInput")
    convb = nc.dram_tensor("convb", [C, 3], F32, kind="ExternalInput")

    flat = None
    if not merged:
        flat = nc.dram_tensor("flat", [C, BL, N, T], BF16, kind="ExternalOutput")
    if DEBUG:
        hT_out = nc.dram_tensor("hT_out", [64, S], F32, kind="ExternalOutput")
        what_out = nc.dram_tensor("what_out", [64, BL * 64], F32, kind="ExternalOutput")
        fb1_out = nc.dram_tensor("fb1_out", [64, S * 32], BF16, kind="ExternalOutput")

    from contextlib import ExitStack
    with tile.TileContext(nc) as tc, ExitStack() as stack:
        cpool = stack.enter_context(tc.tile_pool(name="const", bufs=1))
        perpool = stack.enter_context(tc.tile_pool(name="persist", bufs=1))
        spool = stack.enter_context(tc.tile_pool(name="small", bufs=4))
        if merged:
            dpool = stack.enter_context(tc.tile_pool(name="dram", bufs=1, space="DRAM"))
            wpool = stack.enter_context(tc.tile_pool(name="wts", bufs=9))
            a2a_in0 = dpool.tile([8 * 2 * 8 * C * T], BF16)
            a2a_in1 = dpool.tile([8 * 2 * 8 * C * T], BF16)
            a2a_out0 = dpool.tile([B // 2, FSH], BF16)
            a2a_out1 = dpool.tile([B // 2, FSH], BF16)
            a2a_ins = [a2a_in0, a2a_in1]
            a2a_outs = [a2a_out0, a2a_out1]
        wt_tiles = {}
        fpool = tc.alloc_tile_pool(name="feats", bufs=2)  # featsA only
        # GRU/feats0-only tenants; released before the blocks phase so the
        # weight-stream pool and feature buffers fit together
        epool = tc.alloc_tile_pool(name="early", bufs=1)

        # ---- constants into SBUF
        wrz_sb = epool.tile([65, 128], BF16)
        nc.sync.dma_start(wrz_sb[:], w_rz[:])
        wn_sb = epool.tile([64, 64], BF16)
        nc.sync.dma_start(wn_sb[:], w_n[:])
        wgin_sb = epool.tile([1, 64], BF16)
        nc.sync.dma_start(wgin_sb[:], w_gin[:])
        wih_sb = epool.tile([1, 128], BF16)
        nc.sync.dma_start(wih_sb[:], w_ih[:])
        wqk_sb = cpool.tile([64, 64], F32)
        nc.sync.dma_start(wqk_sb[:], w_qk[:])
        gbias_sb = epool.tile([64, 4], F32)
        nc.sync.dma_start(gbias_sb[:], gbias[:])
        qkb_sb = cpool.tile([QK, 2], F32)
        nc.sync.dma_start(qkb_sb[:], qkb[:])
        idf_sb = cpool.tile([128, 128], F32)
        nc.sync.dma_start(idf_sb[:], ident_f[:])
        idb_sb = cpool.tile([128, 128], BF16)
        nc.sync.dma_start(idb_sb[:], ident_b[:])
        mx_sb = epool.tile([128, 64 * C], BF16)
        nc.sync.dma_start(mx_sb[:], m_x2i[:])
        beta_sb = epool.tile([1, 512], BF16)
        nc.sync.dma_start(beta_sb[:], beta_row[:])
        # gcn weights: (c parts, (blk, t, d))
        gcnw_sb = cpool.tile([C, 3 * T * C], BF16)
        for kk in range(3):
            nc.sync.dma_start(
                gcnw_sb[:, kk * T * C:(kk + 1) * T * C].rearrange(
                    "p (t d) -> p t d", t=T),
                gcnw[kk, :, :, :].rearrange("t c d -> c t d"),
            )
        # gcn bias: (d parts, (blk, t))
        gcnb_sb = cpool.tile([C, 3 * T], F32)
        nc.sync.dma_start(
            gcnb_sb[:].rearrange("p (k t) -> p k t", k=3),
            gcnb[:].rearrange("k t d -> d k t"),
        )
        cwt_sb = cpool.tile([C, 15 * C], BF16)
        nc.sync.dma_start(
            cwt_sb[:].rearrange("p (q e) -> p q e", q=15),
            cwt[:].rearrange("q d e -> d q e"),
        )
        convb_sb = cpool.tile([C, 3], F32)
        nc.sync.dma_start(convb_sb[:], convb[:])
        # x row (1, T*S) in (t, b, n) order (bf16: matmul operand)
        xrow_sb = epool.tile([1, T * S], BF16)
        for b_ in range(BL):
            nc.sync.dma_start(
                xrow_sb[:].rearrange("p (t b n) -> p t b n", t=T, b=BL)[:, :, b_, :].opt(),
                xlocb[b_, :, :],
            )
        # x (bt parts, n) bf16 for feats0
        xbt_sb = epool.tile([128, N], BF16)
        nc.sync.dma_start(
            xbt_sb[:], xlocb[:].rearrange("b t n -> (b t) n")
        )
        ones_b = epool.tile([1, 128], BF16)
        nc.vector.memset(ones_b[:], 1.0)
        ones_f = cpool.tile([128, 1], F32)
        nc.vector.memset(ones_f[:], 1.0)
        onesrow_f = cpool.tile([1, 128], F32)
        nc.vector.memset(onesrow_f[:], 1.0)

        if merged:
            # prefetch exactly bufs weight tiles so they stream during GRU/GCN
            # without head-of-line blocking the SP DMA queue
            for g4 in range(9):
                wt4 = wpool.tile([128, 4 * NR], BF16, tag="wt")
                nc.sync.dma_start(
                    wt4[:].rearrange("p (k r) -> p k r", k=4),
                    wT[512 * g4:512 * (g4 + 1), :].rearrange(
                        "(k p) r -> p k r", p=128),
                )
                wt_tiles[g4] = wt4

        # ---- GRU (x-terms folded in as K=1 accumulating matmuls; no copies)
        h_tile = perpool.tile([64, S], BF16, tag="h")
        nc.vector.memset(h_tile[:], 0.0)
        HW_ = S // 2
        with tc.tile_pool(name="psg", bufs=2, space="PSUM") as psg:
            for t_ in range(GRUT):
                for hs in range(2):
                    sl = slice(HW_ * hs, HW_ * (hs + 1))
                    gsl = slice(t_ * S + HW_ * hs, t_ * S + HW_ * (hs + 1))
                    xr = xrow_sb[:, gsl]
                    pgin = psg.tile([64, HW_], F32, tag="pgin")
                    nc.tensor.matmul(pgin[:], wgin_sb[:], xr,
                                     start=True, stop=True)
                    pr = psg.tile([64, HW_], F32, tag="pr")
                    nc.tensor.matmul(pr[:], wrz_sb[0:64, 0:64], h_tile[:, sl],
                                     start=True, stop=False)
                    nc.tensor.matmul(pr[:], wih_sb[:, 0:64], xr,
                                     start=False, stop=True)
                    pz = psg.tile([64, HW_], F32, tag="pz")
                    nc.tensor.matmul(pz[:], wrz_sb[0:64, 64:128], h_tile[:, sl],
                                     start=True, stop=False)
                    nc.tensor.matmul(pz[:], wih_sb[:, 64:128], xr,
                                     start=False, stop=True)
                    pn = psg.tile([64, HW_], F32, tag="pn")
                    nc.tensor.matmul(pn[:], wn_sb[:], h_tile[:, sl],
                                     start=True, stop=True)

                    r_sb = spool.tile([64, HW_], F32, tag=f"r{hs}")
                    nc.scalar.activation(r_sb[:], pr[:], AF.Sigmoid,
                                         bias=gbias_sb[:, 0:1], scale=1.0)
                    z_sb = spool.tile([64, HW_], BF16, tag=f"z{hs}")
                    nc.scalar.activation(z_sb[:], pz[:], AF.Sigmoid,
                                         bias=gbias_sb[:, 1:2], scale=1.0)
                    # off-chain: w = 1 - z (gpsimd), q1 = z * h (gpsimd)
                    w_sb = spool.tile([64, HW_], F32, tag=f"w{hs}")
                    nc.gpsimd.tensor_scalar(w_sb[:], z_sb[:], -1.0, 1.0,
                                            op0=ALU.mult, op1=ALU.add)
                    q1 = spool.tile([64, HW_], F32, tag=f"q1{hs}")
                    nc.gpsimd.tensor_tensor(q1[:], z_sb[:], h_tile[0:64, sl],
                                            op=ALU.mult)
                    # chain: t1 = (pn + bhh_n) * r ; t2 = t1 + gin ; n = tanh
                    t1 = spool.tile([64, HW_], F32, tag=f"t1{hs}")
                    nc.vector.scalar_tensor_tensor(
                        t1[:], pn[:], gbias_sb[:, 2:3], r_sb[:],
                        op0=ALU.add, op1=ALU.mult
                    )
                    t2 = spool.tile([64, HW_], F32, tag=f"t2{hs}")
                    nc.vector.tensor_tensor(t2[:], t1[:], pgin[:], op=ALU.add)
                    n_sb = spool.tile([64, HW_], F32, tag=f"n{hs}")
                    nc.scalar.activation(n_sb[:], t2[:], AF.Tanh,
                                         bias=gbias_sb[:, 3:4], scale=1.0)
                    # h' = (1-z)*n + z*h = n*w + q1
                    q2 = spool.tile([64, HW_], F32, tag=f"q2{hs}")
                    nc.vector.tensor_tensor(q2[:], n_sb[:], w_sb[:], op=ALU.mult)
                    nc.vector.tensor_tensor(h_tile[0:64, sl], q2[:], q1[:],
                                            op=ALU.add)

        hf_sb = perpool.tile([64, S], F32, tag="hf")
        nc.scalar.copy(hf_sb[:], h_tile[:])
        if DEBUG:
            nc.sync.dma_start(hT_out[:], hf_sb[:])

        # ---- Q/K + adjacency
        what_sb = perpool.tile([64, BL * 64], BF16, tag="what")
        with tc.tile_pool(name="psa", bufs=1, space="PSUM") as psa:
            pq = psa.tile([QK, S], F32, tag="pq")
            nc.tensor.matmul(pq[:], wqk_sb[:, 0:QK], hf_sb[:], start=True, stop=True)
            pk = psa.tile([QK, S], F32, tag="pk")
            nc.tensor.matmul(pk[:], wqk_sb[:, QK:2 * QK], hf_sb[:], start=True, stop=True)
            q_sb = perpool.tile([QK, S], F32, tag="q")
            nc.scalar.activation(q_sb[:], pq[:], AF.Identity,
                                 bias=qkb_sb[:, 0:1], scale=1.0)
            k_sb = perpool.tile([QK, S], F32, tag="k")
            nc.scalar.activation(k_sb[:], pk[:], AF.Identity,
                                 bias=qkb_sb[:, 1:2], scale=1.0)

            for b_ in range(BL):
                ps = psa.tile([64, 64], F32, tag="ps")
                nc.tensor.matmul(
                    ps[:],
                    q_sb[:, b_ * 64:(b_ + 1) * 64],
                    k_sb[:, b_ * 64:(b_ + 1) * 64],
                    start=True, stop=True,
                )
                m_sb = spool.tile([64, 1], F32, tag="m")
                nc.vector.tensor_reduce(m_sb[:], ps[:], axis=mybir.AxisListType.X, op=ALU.max)
                negm = spool.tile([64, 1], F32, tag="negm")
                nc.vector.tensor_scalar_mul(negm[:], m_sb[:], -(QK ** -0.5))
                e_sb = spool.tile([64, 64], F32, tag="e")
                rowsum = spool.tile([64, 1], F32, tag="rowsum")
                nc.scalar.activation(
                    e_sb[:], ps[:], AF.Exp, bias=negm[:], scale=QK ** -0.5,
                    accum_out=rowsum[:],
                )
                rinv = spool.tile([64, 1], F32, tag="rinv")
                nc.vector.reciprocal(rinv[:], rowsum[:])
                wadj_sb = spool.tile([64, 64], F32, tag="wadj")
                nc.vector.tensor_scalar_mul(wadj_sb[:], e_sb[:], rinv[:])
                pdeg = psa.tile([64, 1], F32, tag="pdeg")
                nc.tensor.matmul(pdeg[:], wadj_sb[:], ones_f[0:64, :], start=True, stop=True)
                sdeg = spool.tile([64, 1], F32, tag="sdeg")
                nc.scalar.sqrt(sdeg[:], pdeg[:])
                dinv = spool.tile([64, 1], F32, tag="dinv")
                nc.vector.reciprocal(dinv[:], sdeg[:])
                rs = spool.tile([64, 1], F32, tag="rs")
                nc.vector.tensor_tensor(rs[:], rinv[:], dinv[:], op=ALU.mult)
                pt = psa.tile([1, 64], F32, tag="pt")
                nc.tensor.transpose(pt[:], dinv[:], idf_sb[0:64, 0:64])
                drow = spool.tile([1, 64], F32, tag="drow")
                nc.scalar.copy(drow[:], pt[:])
                pbc = psa.tile([64, 64], F32, tag="pbc")
                nc.tensor.matmul(pbc[:], onesrow_f[0:1, 0:64], drow[:], start=True, stop=True)
                nc.vector.scalar_tensor_tensor(
                    what_sb[:, b_ * 64:(b_ + 1) * 64], e_sb[:], rs[:], pbc[:],
                    op0=ALU.mult, op1=ALU.mult,
                )

        if DEBUG:
            wtmp = perpool.tile([64, BL * 64], F32, tag="wtmp")
            nc.vector.tensor_copy(wtmp[:], what_sb[:])
            nc.sync.dma_start(what_out[:], wtmp[:])

        # ---- feats0: (n, (b, t, c)) = x * w_x2i + b_x2i
        # groups of 32 (b,t) pairs; mx = kron(I32, w_x2i) replicated 4x on parts
        featsA = fpool.tile([64, S * 32], BF16, tag="featsA")
        with tc.tile_pool(name="psf", bufs=4, space="PSUM") as psf:
            for g in range(2):
                for j4 in range(8):
                    j = g * 8 + j4
                    pf = psf.tile([64, 512], F32, tag="pf")
                    nc.tensor.matmul(
                        pf[:], xbt_sb[64 * g:64 * (g + 1), :],
                        mx_sb[64 * g:64 * (g + 1), 512 * j4:512 * (j4 + 1)],
                        start=True, stop=False)
                    nc.tensor.matmul(pf[:], ones_b[0:1, 0:64], beta_sb[:],
                                     start=False, stop=True)
                    if j % 2 == 0:
                        nc.scalar.copy(featsA[:, 512 * j:512 * (j + 1)], pf[:])
                    else:
                        nc.vector.tensor_copy(featsA[:, 512 * j:512 * (j + 1)], pf[:])

        epool.release()
        fpoolD = tc.alloc_tile_pool(name="featsD", bufs=2)
        fpoolB = tc.alloc_tile_pool(name="featsB", bufs=1)

        # ---- blocks
        sbufC = perpool.tile([64, S * TSLOT], BF16, tag="sbufC")  # (d,(b,j,slot))
        cview = sbufC[:].rearrange("p (bj s) -> p bj s", s=TSLOT)
        nc.vector.memset(cview[:, :, 0:PAD], 0.0)
        nc.vector.memset(cview[:, :, TSLOT - PAD:TSLOT], 0.0)

        tap0 = [0, 3, 8]
        with tc.tile_pool(name="psb", bufs=2, space="PSUM") as psb:
            for blk in range(NBLK):
                k = KS[blk]
                p = k // 2
                # agg: out1 (c, j) per (t,b): sbufB (c, (t, b, j))
                sbufB = fpoolB.tile([64, S * 32], BF16, tag="sbufB")
                for g in range(16):
                    p1 = psb.tile([64, 512], F32, tag="p1")
                    for m in range(8):
                        tb = g * 8 + m
                        t_, b_ = tb // BL, tb % BL
                        nc.tensor.matmul(
                            p1[:, 64 * m:64 * (m + 1)],
                            featsA[:, (b_ * T + t_) * 64:(b_ * T + t_ + 1) * 64],
                            what_sb[:, b_ * 64:(b_ + 1) * 64],
                            start=True, stop=True,
                        )
                    if g % 2 == 0:
                        nc.scalar.copy(sbufB[:, 512 * g:512 * (g + 1)], p1[:])
                    else:
                        nc.vector.tensor_copy(sbufB[:, 512 * g:512 * (g + 1)], p1[:])

                # theta + bias -> sbufC (d, (b, j, slot))
                for t_ in range(T):
                    p2 = psb.tile([64, 256], F32, tag="p2")
                    nc.tensor.matmul(
                        p2[:],
                        gcnw_sb[:, (blk * T + t_) * 64:(blk * T + t_ + 1) * 64],
                        sbufB[:, (t_ * BL) * 64:(t_ * BL + BL) * 64],
                        start=True, stop=True,
                    )
                    outv = cview[:, :, PAD + t_:PAD + t_ + 1].opt()
                    if t_ % 4 != 3:
                        nc.vector.tensor_scalar(
                            outv, p2[:], 1.0,
                            gcnb_sb[:, blk * T + t_:blk * T + t_ + 1],
                            op0=ALU.mult, op1=ALU.add,
                        )
                    else:
                        nc.scalar.activation(
                            outv, p2[:], AF.Identity,
                            bias=gcnb_sb[:, blk * T + t_:blk * T + t_ + 1], scale=1.0,
                        )

                # conv + bias + leaky -> sbufD (e, (b, n, t))
                sbufD = fpoolD.tile([64, S * 32], BF16, tag="sbufD")
                cv4 = sbufC[:].rearrange("p (b j s) -> p b j s", b=BL, j=64)
                dv3c = sbufD[:].rearrange("p (b n t) -> p b n t", b=BL, n=N)
                for b_ in range(BL):
                    for jg in range(4):
                        p3 = psb.tile([64, 512], F32, tag="p3")
                        for dt in range(k):
                            s0 = PAD - p + dt
                            rhs = cv4[:, b_:b_ + 1, jg * 16:(jg + 1) * 16, s0:s0 + T].opt()
                            nc.tensor.matmul(
                                p3[:],
                                cwt_sb[:, (tap0[blk] + dt) * 64:(tap0[blk] + dt + 1) * 64],
                                rhs,
                                start=(dt == 0), stop=(dt == k - 1),
                            )
                        nc.scalar.activation(
                            sbufD[:, (b_ * 64 + jg * 16) * 32:(b_ * 64 + jg * 16 + 16) * 32],
                            p3[:], AF.Lrelu,
                            bias=convb_sb[:, blk:blk + 1], scale=1.0, alpha=0.01,
                        )
                    if merged and blk == 2 and b_ % 2 == 1:
                        hh = b_ // 2
                        av5 = a2a_ins[hh][:].rearrange(
                            "(s bl n8 e t) -> s bl n8 e t", s=8, bl=2, n8=8, e=C)
                        for nb in range(8):
                            for bl in range(2):
                                nc.sync.dma_start(
                                    av5[nb, bl, :, :, :].rearrange("n8 e t -> e n8 t"),
                                    dv3c[:, 2 * hh + bl, 8 * nb:8 * (nb + 1), :],
                                )
                        nc.gpsimd.collective_compute(
                            "AllToAll", ALU.bypass,
                            replica_groups=[list(range(NCORES))],
                            ins=[a2a_ins[hh].opt()], outs=[a2a_outs[hh].opt()],
                        )

                if blk < 2:
                    # transpose (e,n) -> (n,e) per (b,t): featsA_next (n,(b,t,c))
                    featsA = fpool.tile([64, S * 32], BF16, tag="featsA")
                    dv = sbufD[:].rearrange("p (b n t) -> p b n t", b=BL, n=N)
                    for g in range(16):
                        p4 = psb.tile([64, 512], BF16, tag="p4")
                        for m in range(8):
                            bt = g * 8 + m
                            b_, t_ = bt // T, bt % T
                            inv = dv[:, b_:b_ + 1, :, t_:t_ + 1].opt()
                            nc.tensor.transpose(
                                p4[:, 64 * m:64 * (m + 1)], inv, idb_sb[0:64, 0:64]
                            )
                        if g % 2 == 0:
                            nc.scalar.copy(featsA[:, 512 * g:512 * (g + 1)], p4[:])
                        else:
                            nc.vector.tensor_copy(featsA[:, 512 * g:512 * (g + 1)], p4[:])
                    if blk == 0 and DEBUG:
                        nc.sync.dma_start(fb1_out[:], featsA[:])
                else:
                    if not merged:
                        nc.sync.dma_start(
                            flat[:].rearrange("e b n t -> e (b n t)"), sbufD[:]
                        )

        fpoolB.release()
        fpoolD.release()
        fpool.release()
        if merged:
            KT = FSH // 128
            with (
                tc.tile_pool(name="lout", bufs=1) as lpool,
                tc.tile_pool(name="psl", bufs=2, space="PSUM") as psl,
            ):
                actT_sb = lpool.tile([128, KT * B], BF16)
                atv = actT_sb[:].rearrange("p (kt c) -> p kt c", c=B)
                for hh in range(2):
                    a_sb = lpool.tile([B // 2, FSH], BF16, tag=f"act{hh}")
                    nc.sync.dma_start(a_sb[:], a2a_outs[hh][:])
                    for g in range(8):
                        p5 = psl.tile([128, 256], BF16, tag="p5")
                        for m in range(16):
                            kt = g * 16 + m
                            nc.tensor.transpose(
                                p5[:, 16 * m:16 * (m + 1)],
                                a_sb[:, 128 * kt:128 * (kt + 1)],
                                idb_sb[0:16, 0:16],
                            )
                        outv = atv[:, g * 16:(g + 1) * 16,
                                   16 * hh:16 * (hh + 1)].opt()
                        if g % 2 == 0:
                            nc.scalar.copy(outv, p5[:])
                        else:
                            nc.vector.tensor_copy(outv, p5[:])

                psA = psl.tile([B, 512], F32, tag="psA")
                psB = psl.tile([B, 256], F32, tag="psB")
                for g4 in range(KT // 4):
                    if g4 in wt_tiles:
                        wt4 = wt_tiles[g4]
                    else:
                        wt4 = wpool.tile([128, 4 * NR], BF16, tag="wt")
                        nc.sync.dma_start(
                            wt4[:].rearrange("p (k r) -> p k r", k=4),
                            wT[512 * g4:512 * (g4 + 1), :].rearrange(
                                "(k p) r -> p k r", p=128),
                        )
                    for kk in range(4):
                        kt = g4 * 4 + kk
                        lhsT = actT_sb[:, kt * B:(kt + 1) * B]
                        nc.tensor.matmul(psA[:], lhsT, wt4[:, kk * NR:kk * NR + 512],
                                         start=(kt == 0), stop=(kt == KT - 1))
                        nc.tensor.matmul(psB[:], lhsT,
                                         wt4[:, kk * NR + 512:(kk + 1) * NR],
                                         start=(kt == 0), stop=(kt == KT - 1))
                out_sb = lpool.tile([B, NR], F32)
                nc.scalar.copy(out_sb[:, 0:512], psA[:])
                nc.scalar.copy(out_sb[:, 512:NR], psB[:])
                nc.sync.dma_start(partial[:], out_sb[:])

    nc.compile()
    return nc


# ---------------------------------------------------------------- launch 2
def build_launch2():
    nc = bacc.Bacc("TRN2", target_bir_lowering=False, num_devices=NCORES)
    actT = nc.dram_tensor("actT", [FSH, B], BF16, kind="ExternalInput")
    wT = nc.dram_tensor("wT", [FSH, NR], BF16, kind="ExternalInput")
    partial = nc.dram_tensor("partial", [B, NR], F32, kind="ExternalOutput")

    KT = FSH // 128  # 128 k-tiles
    with tile.TileContext(nc) as tc:
        with (
            tc.tile_pool(name="acts", bufs=1) as apool,
            tc.tile_pool(name="wts", bufs=8) as wpool,
            tc.tile_pool(name="outs", bufs=1) as opool,
            tc.tile_pool(name="ps", bufs=1, space="PSUM") as ppool,
        ):
            act_sb = apool.tile([128, KT * B], BF16)
            nc.sync.dma_start(
                act_sb[:].rearrange("p (k b) -> p k b", b=B),
                actT[:].rearrange("(k p) b -> p k b", p=128),
            )
            psA = ppool.tile([B, 512], F32, tag="psA")
            psB = ppool.tile([B, 256], F32, tag="psB")
            for g4 in range(KT // 4):
                wt4 = wpool.tile([128, 4 * NR], BF16, tag="wt")
                nc.sync.dma_start(
                    wt4[:].rearrange("p (k r) -> p k r", k=4),
                    wT[512 * g4:512 * (g4 + 1), :].rearrange("(k p) r -> p k r", p=128),
                )
                for kk in range(4):
                    kt = g4 * 4 + kk
                    lhsT = act_sb[:, kt * B:(kt + 1) * B]
                    nc.tensor.matmul(psA[:], lhsT, wt4[:, kk * NR:kk * NR + 512],
                                     start=(kt == 0), stop=(kt == KT - 1))
                    nc.tensor.matmul(psB[:], lhsT, wt4[:, kk * NR + 512:(kk + 1) * NR],
                                     start=(kt == 0), stop=(kt == KT - 1))
            out_sb = opool.tile([B, NR], F32)
            nc.scalar.copy(out_sb[:, 0:512], psA[:])
            nc.scalar.copy(out_sb[:, 512:NR], psB[:])
            nc.sync.dma_start(partial[:], out_sb[:])

    nc.compile()
    return nc


# ---------------------------------------------------------------- host glue
def _prep_shared(inp):
    f32 = np.float32
    whh = np.asarray(inp["gru_whh"], f32)      # (192, 64)
    wih = np.asarray(inp["gru_wih"], f32)[:, 0]  # (192,)
    bih = np.asarray(inp["gru_bih"], f32)
    bhh = np.asarray(inp["gru_bhh"], f32)
    H = GRU_H
    w_rz = np.zeros((65, 128), f32)
    w_rz[0:64, :] = whh[0:2 * H, :].T
    w_rz[64, :] = wih[0:2 * H]
    w_rz = w_rz.astype(BF)
    w_n = np.ascontiguousarray(whh[2 * H:, :].T).astype(BF)
    w_gin = wih[2 * H:][None, :].astype(BF)  # (1, 64)
    w_ih = wih[0:2 * H][None, :].astype(BF)  # (1, 128)
    gbias = np.zeros((64, 4), f32)
    gbias[:, 0] = bih[0:H] + bhh[0:H]
    gbias[:, 1] = bih[H:2 * H] + bhh[H:2 * H]
    gbias[:, 2] = bhh[2 * H:]
    gbias[:, 3] = bih[2 * H:]
    wq_w = np.asarray(inp["wq_w"], f32); wq_b = np.asarray(inp["wq_b"], f32)
    wk_w = np.asarray(inp["wk_w"], f32); wk_b = np.asarray(inp["wk_b"], f32)
    w_qk = np.zeros((64, 64), f32)
    w_qk[:, 0:QK] = wq_w.T
    w_qk[:, QK:] = wk_w.T
    qkb = np.stack([wq_b, wk_b], axis=1).astype(f32)  # (32, 2)
    ident = np.eye(128, dtype=f32)
    w2i = np.asarray(inp["w_x2i"], f32)
    b2i = np.asarray(inp["b_x2i"], f32)
    m64 = np.kron(np.eye(64, dtype=f32), w2i[None, :])  # (64, 4096)
    m_x2i = np.tile(m64, (2, 1)).astype(BF)  # (128, 4096) 2x replicated
    beta_row = np.tile(b2i, 8)[None, :].astype(BF)  # (1, 512)
    gcnw = np.stack([np.asarray(inp[f"gcn_w{i}"], f32) for i in range(3)]).astype(BF)
    gcnb = np.stack([np.asarray(inp[f"gcn_b{i}"], f32) for i in range(3)]).astype(f32)
    cwt = np.concatenate(
        [np.asarray(inp[f"conv_w{i}"], f32).transpose(2, 1, 0) for i in range(3)]
    ).astype(BF)  # (15, d, e)
    convb = np.stack([np.asarray(inp[f"conv_b{i}"], f32) for i in range(3)], axis=1)
    return {
        "w_rz": w_rz, "w_n": w_n, "w_gin": w_gin, "w_ih": w_ih, "w_qk": w_qk,
        "gbias": gbias, "qkb": qkb,
        "ident_f": ident, "ident_b": ident.astype(BF),
        "m_x2i": m_x2i, "beta_row": beta_row,
        "gcnw": gcnw, "gcnb": gcnb, "cwt": cwt, "convb": convb,
    }


_NC_CACHE = {}


def _get_nc(name, builder):
    if name not in _NC_CACHE:
        _NC_CACHE[name] = builder()
    return _NC_CACHE[name]


MERGED = not bool(int(os.environ.get("KTWOLAUNCH", "0")))


def kernel(**inputs):
    global LAST_EXEC_NS, LAST_RESULTS
    LAST_EXEC_NS = []
    LAST_RESULTS = []
    inp = {k: np.asarray(v) for k, v in inputs.items()}
    shared = _prep_shared(inp)
    x = np.asarray(inp["x"], np.float32)

    if MERGED:
        nc1 = _get_nc("m", lambda: build_launch1(merged=True))
        lw = np.asarray(inp["lout_w"], np.float32).reshape(NR, N, C, T)
        in_maps = []
        for i in range(NCORES):
            xl = np.ascontiguousarray(x[BL * i:BL * (i + 1)])
            m = dict(shared)
            m["xloc"] = xl
            m["xlocb"] = xl.astype(BF)
            m["wT"] = np.ascontiguousarray(
                lw[:, 8 * i:8 * (i + 1)].reshape(NR, FSH).T.astype(BF))
            in_maps.append(m)
        r1 = run_bass_kernel_spmd(nc1, in_maps, core_ids=list(range(NCORES)))
        LAST_RESULTS.append(r1)
        LAST_EXEC_NS.append(r1.exec_time_ns)
        psum_ = np.zeros((B, NR), np.float32)
        for j in range(NCORES):
            psum_ += np.asarray(r1.results[j]["partial"])
        # partial row c corresponds to global batch 4*((c%16)//2)+2*(c//16)+c%2
        perm = np.array([4 * ((c % 16) // 2) + 2 * (c // 16) + (c % 2)
                         for c in range(B)])
        out = np.zeros((B, NR), np.float32)
        out[perm] = psum_
        out += np.asarray(inp["lout_b"], np.float32)
        return out.reshape(B, HOR, N).astype(np.float32)

    nc1 = _get_nc("l1", build_launch1)
    in_maps = []
    for i in range(NCORES):
        xl = np.ascontiguousarray(x[BL * i:BL * (i + 1)])
        m = dict(shared)
        m["xloc"] = xl
        m["xlocb"] = xl.astype(BF)
        in_maps.append(m)
    r1 = run_bass_kernel_spmd(nc1, in_maps, core_ids=list(range(NCORES)))
    LAST_RESULTS.append(r1)
    LAST_EXEC_NS.append(r1.exec_time_ns)

    # assemble act (B, N, T, C): flat_i is (c, b_local, n, t)
    act = np.zeros((B, N, T, C), BF)
    for i in range(NCORES):
        fl = np.asarray(r1.results[i]["flat"])  # (C, BL, N, T)
        act[BL * i:BL * (i + 1)] = fl.transpose(1, 2, 3, 0)
    act2 = act.reshape(B, FEAT)  # feature order (n, t, c)

    # l_out weight: cols reordered (n, c, t) -> (n, t, c)
    lw = np.asarray(inp["lout_w"], np.float32).reshape(NR, N, C, T)
    lw = lw.transpose(0, 1, 3, 2).reshape(NR, FEAT)

    nc2 = _get_nc("l2", build_launch2)
    in_maps2 = []
    for j in range(NCORES):
        sl = slice(FSH * j, FSH * (j + 1))
        in_maps2.append({
            "actT": np.ascontiguousarray(act2[:, sl].T),
            "wT": np.ascontiguousarray(lw[:, sl].T.astype(BF)),
        })
    r2 = run_bass_kernel_spmd(nc2, in_maps2, core_ids=list(range(NCORES)))
    LAST_RESULTS.append(r2)
    LAST_EXEC_NS.append(r2.exec_time_ns)

    out = np.zeros((B, NR), np.float32)
    for j in range(NCORES):
        out += np.asarray(r2.results[j]["partial"])
    out += np.asarray(inp["lout_b"], np.float32)
    return out.reshape(B, HOR, N).astype(np.float32)

